# revision 21
# baseline (speedup 1.0000x reference)
"""DiagonalLinear: y = x * w + b (elementwise over features).

x: (16384, 4096) f32, w: (4096,) f32, b: (4096,) f32.

Sharding: data-parallel over the batch dim across 8 NeuronCores (2048 rows
each), weight/bias replicated — fully elementwise, no collectives.

Per-core kernel (Tile framework, one NEFF run SPMD on cores 0-7):
  - w|b packed host-side into one [1, 8192] tensor, DMA'd (32 KiB) into
    partition 0 of the const tile, then broadcast across all 128
    partitions OFF the DMA fabric: a K=1 fp32 PE matmul against a ones
    row (bit-exact on TRN2: 1.0*w) writes PSUM chunks that ACT copies
    back over the const tile. Keeps the saturated 16-SDMA fabric for x/y
    traffic only (the mandatory 64 MiB/core), and building consts in
    place avoids a second 32 KiB/partition SBUF column.
  - x-tile 0 is processed as four 1 MiB chunks: under fair-share DMA the
    first completion scales with co-queued bytes, so small first chunks
    start the vector engine ~3x earlier.
  - Tiles 1-7 are [128, 2*4096] (4 MiB per DMA): load on the SP HWDGE
    ring, DVE fp32 mul+add in place (bit-exact vs the reference), store
    on the ACT HWDGE ring; 3-slot main pool + 4-slot warmup pool.

The kernel is DMA-fabric/DVE co-limited: 64 MiB/core at ~425 GB/s
effective (~155 us) overlapping 141 us of DVE tensor_tensor work;
~183-215 us wall depending on neighbor load on the shared device.
"""

import numpy as np

import concourse.bacc as bacc
import concourse.mybir as mybir
import concourse.tile as tile
from concourse.bass_utils import run_bass_kernel_spmd

N_CORES = 8
BATCH = 16384
D = 4096
ROWS_PER_CORE = BATCH // N_CORES  # 2048
P = 128

Q = 2            # 128-row blocks per main tile -> 4 MiB DMAs
MAIN_BUFS = 3
WARM_CHUNKS = 4  # x-tile 0 split into 1 MiB chunks
MM_N = 512       # one PSUM bank per broadcast matmul

_CACHE = {}


def build_nc(q=Q, main_bufs=MAIN_BUFS, warm_chunks=WARM_CHUNKS):
    nc = bacc.Bacc()
    f32 = mybir.dt.float32
    x = nc.dram_tensor("x", [ROWS_PER_CORE, D], f32, kind="ExternalInput")
    wb_in = nc.dram_tensor("wb", [1, 2 * D], f32, kind="ExternalInput")
    y = nc.dram_tensor("y", [ROWS_PER_CORE, D], f32, kind="ExternalOutput")

    n_tiles = ROWS_PER_CORE // (P * q)
    assert n_tiles * P * q == ROWS_PER_CORE
    C = q * D // warm_chunks

    # tile n, partition p, free (j, d) <-> row n*(q*P) + j*P + p, col d
    x_r = x.rearrange("(n j p) d -> n p j d", p=P, j=q)
    y_r = y.rearrange("(n j p) d -> n p j d", p=P, j=q)

    with tile.TileContext(nc) as tc:
        with (
            tc.tile_pool(name="consts", bufs=1) as cpool,
            tc.tile_pool(name="warm", bufs=warm_chunks) as wpool,
            tc.tile_pool(name="work", bufs=main_bufs) as pool,
            tc.tile_pool(name="psum", bufs=4, space="PSUM") as ppool,
        ):
            consts = cpool.tile([P, 2 * D], f32)  # [:, :D]=w, [:, D:]=b
            ones = cpool.tile([1, P], f32)
            with tc.high_priority():
                nc.scalar.dma_start(consts[0:1, :], wb_in[:, :])
                nc.gpsimd.memset(ones[:, :], 1.0)
                for k in range(2 * D // MM_N):
                    pt = ppool.tile([P, MM_N], f32)
                    nc.tensor.matmul(
                        pt[:, :], ones[:, :], consts[0:1, k * MM_N : (k + 1) * MM_N],
                        start=True, stop=True,
                    )
                    nc.scalar.copy(consts[:, k * MM_N : (k + 1) * MM_N], pt[:, :])

            wt = consts[:, 0:D]
            bt = consts[:, D : 2 * D]
            # warmup: x-tile 0 in small chunks so DVE starts early
            for c in range(warm_chunks):
                j, f0 = (c * C) // D, (c * C) % D
                tw = wpool.tile([P, C], f32)
                nc.sync.dma_start(tw[:, :], x_r[0][:, j, f0 : f0 + C])
                nc.vector.tensor_mul(tw[:, :], tw[:, :], wt[:, f0 : f0 + C])
                nc.vector.tensor_add(tw[:, :], tw[:, :], bt[:, f0 : f0 + C])
                nc.scalar.dma_start(y_r[0][:, j, f0 : f0 + C], tw[:, :])
            for i in range(1, n_tiles):
                t = pool.tile([P, q * D], f32)
                t3 = t[:, :].rearrange("p (j d) -> p j d", j=q)
                nc.sync.dma_start(t3, x_r[i])
                for j in range(q):
                    s = t[:, j * D : (j + 1) * D]
                    nc.vector.tensor_mul(s, s, wt)
                    nc.vector.tensor_add(s, s, bt)
                nc.scalar.dma_start(y_r[i], t3)
    nc.compile()
    return nc


def _get_nc():
    if "nc" not in _CACHE:
        _CACHE["nc"] = build_nc()
    return _CACHE["nc"]


def run(input, weight, bias, nc=None, **spmd_kwargs):
    if nc is None:
        nc = _get_nc()
    x = np.ascontiguousarray(input, dtype=np.float32)
    wb = np.ascontiguousarray(
        np.stack([np.asarray(weight), np.asarray(bias)]).astype(np.float32)
    ).reshape(1, 2 * D)
    in_maps = [
        {"x": x[c * ROWS_PER_CORE : (c + 1) * ROWS_PER_CORE], "wb": wb}
        for c in range(N_CORES)
    ]
    res = run_bass_kernel_spmd(nc, in_maps, core_ids=list(range(N_CORES)), **spmd_kwargs)
    out = np.concatenate([r["y"] for r in res.results], axis=0)
    return out, res


def kernel(input, weight, bias):
    out, _ = run(input, weight, bias)
    return out
